# revision 55
# baseline (speedup 1.0000x reference)
"""Trainium2 Bass kernel for a 3-layer GATv2 network + MLP head.

Hardcoded for: N=50000 nodes, E=800000 edges (+N self loops), F=256 input
features, conv1 256->64x8, conv2 512->32x4, conv3 128->16x1, fc 16->8->2.

Sharding: dst-range sharding over 8 NeuronCores. Core k owns nodes
[k*6250, (k+1)*6250) and all edges pointing into that range. Each layer
runs as two SPMD launches: a matmul launch (node-sharded feature
transform) and an edge launch (gather + attention + one-hot-matmul
scatter-add aggregation). The host concatenates per-core shards between
launches (xl must be fully replicated for src gathers).
"""

import numpy as np

import concourse.bacc as bacc
import concourse.mybir as mybir
from concourse.tile import TileContext
from concourse.bass_utils import run_bass_kernel_spmd
from concourse.masks import make_identity

F32 = mybir.dt.float32
F16 = mybir.dt.float16
I16 = mybir.dt.int16

NCORES = 8
N = 50000
NSH = N // NCORES          # 6250 nodes per core
BLK = 128                  # dst nodes per block
NBLK = (NSH + BLK - 1) // BLK   # 49
NPAD = NBLK * BLK          # 6272 padded rows per core shard
CH = 512                   # edges per slot
NEG_SLOPE = 0.2

LAYERS = [
    # (f_in, H, C, hc, hcg, gdt)  hcg = gather-padded row width; gdt = gather dtype
    # gather row bytes must be a multiple of 256
    (256, 8, 64, 512, 512, F16),
    (512, 4, 32, 128, 128, F16),
    (128, 1, 16, 16, 64, F32),
]


# ----------------------------------------------------------------------------
# Host-side graph preprocessing
# ----------------------------------------------------------------------------

def prep_edges(edge_index):
    """Build per-core edge slot arrays.

    Returns dict with per-core arrays and shared slot bases.
    """
    src = np.concatenate([edge_index[0], np.arange(N, dtype=np.int64)]).astype(np.int64)
    dst = np.concatenate([edge_index[1], np.arange(N, dtype=np.int64)]).astype(np.int64)

    core = dst // NSH
    blk = (dst - core * NSH) // BLK
    dstloc = (dst - core * NSH) % BLK

    # group edges by (core, block), sort by src within the group
    order = np.lexsort((src, blk, core))
    src_s, core_s, blk_s, dstloc_s = src[order], core[order], blk[order], dstloc[order]

    # counts per (core, block)
    counts = np.zeros((NCORES, NBLK), dtype=np.int64)
    np.add.at(counts, (core_s, blk_s), 1)
    s_blk = int(np.ceil(counts.max() / CH))  # slots per block
    nslot = NBLK * s_blk

    # per-core padded slot arrays (-1 src marks padding)
    srcs = np.full((NCORES, nslot, CH), -1, dtype=np.int64)
    dloc = np.full((NCORES, nslot, CH), 999.0, dtype=np.float32)
    xr_row = np.zeros((NCORES, nslot, CH), dtype=np.int64)  # row in xr shard

    # prefix offsets into the sorted arrays
    flat_counts = counts.reshape(-1)
    offs = np.concatenate([[0], np.cumsum(flat_counts)])
    for k in range(NCORES):
        for b in range(NBLK):
            i0 = offs[k * NBLK + b]
            cnt = counts[k, b]
            e_src = src_s[i0:i0 + cnt]
            e_dl = dstloc_s[i0:i0 + cnt]
            xr_row[k, b * s_blk:(b + 1) * s_blk, :] = b * BLK
            for s in range(s_blk):
                p = b * s_blk + s
                lo, hi = s * CH, min((s + 1) * CH, cnt)
                if hi > lo:
                    n = hi - lo
                    srcs[k, p, :n] = e_src[lo:hi]
                    dloc[k, p, :n] = e_dl[lo:hi]
                    xr_row[k, p, :n] = b * BLK + e_dl[lo:hi]

    # group slots into gather units of up to 2 slots (1024 edges per gather)
    groups = []
    s = 0
    while s < s_blk:
        ns = min(2, s_blk - s)
        groups.append((s, ns))
        s += ns

    # shared per-gather-unit base (same for all cores; compile-time constant),
    # computed over real edges only; pads get relative index 0.
    valid = srcs >= 0
    smin = np.where(valid, srcs, np.int64(1 << 60))
    base = np.zeros(nslot, dtype=np.int64)  # indexed by slot; equal within a unit
    for b in range(NBLK):
        for (s0, ns) in groups:
            sl = slice(b * s_blk + s0, b * s_blk + s0 + ns)
            m = smin[:, sl, :].min()
            base[sl] = 0 if m == (1 << 60) else m
    rel = np.where(valid, srcs - base[None, :, None], 0)
    assert rel.min() >= 0 and rel.max() < 32768, (rel.min(), rel.max())

    def wrap16(a):
        # [nslot, CH] int -> [128, nslot*CH//16] int16 wrap layout, 8x replicated
        w = a.reshape(a.shape[0], CH // 16, 16).transpose(2, 0, 1).reshape(16, -1)
        return np.tile(w, (8, 1)).astype(np.int16)

    src16 = np.stack([wrap16(rel[k]) for k in range(NCORES)])      # [8,128,nslot*32]
    assert xr_row.max() < 32768
    dst16 = np.stack([wrap16(xr_row[k]) for k in range(NCORES)])   # [8,128,nslot*32]

    # dstloc in gather data layout: edge i -> [i%128, i//128]
    dl = dloc.reshape(NCORES, nslot, CH // 128, 128).transpose(0, 3, 1, 2)
    dl = np.ascontiguousarray(dl.reshape(NCORES, 128, nslot * (CH // 128)), dtype=np.float32)

    return dict(s_blk=s_blk, nslot=nslot, base=base, src16=src16, dst16=dst16,
                dstloc=dl, groups=groups)


# ----------------------------------------------------------------------------
# Device kernels
# ----------------------------------------------------------------------------

def build_mm(f_in, hc_out, odt=F32):
    """xl_sh = hT_sh.T @ Wl ; xr_sh = hT_sh.T @ Wr   (node-sharded, f16 in)."""
    nc = bacc.Bacc("TRN2", target_bir_lowering=False, debug=False, num_devices=NCORES)
    kt = f_in // 128
    hT = nc.dram_tensor("hT", [f_in, NPAD], F16, kind="ExternalInput")
    wl = nc.dram_tensor("wl", [f_in, hc_out], F16, kind="ExternalInput")
    wr = nc.dram_tensor("wr", [f_in, hc_out], F16, kind="ExternalInput")
    xl = nc.dram_tensor("xl", [NPAD, hc_out], odt, kind="ExternalOutput")
    xr = nc.dram_tensor("xr", [NPAD, hc_out], odt, kind="ExternalOutput")

    with TileContext(nc) as tc:
        with (
            tc.tile_pool(name="big", bufs=1) as big,
            tc.tile_pool(name="sb", bufs=3) as sb,
            tc.tile_pool(name="ps", bufs=4, space="PSUM") as ps,
        ):
            hT_sb = big.tile([128, kt, NPAD], F16)
            nc.sync.dma_start(hT_sb[:], hT[:].rearrange("(k p) n -> p k n", p=128))
            wl_sb = big.tile([128, kt, hc_out], F16)
            nc.sync.dma_start(wl_sb[:], wl[:].rearrange("(k p) n -> p k n", p=128))
            wr_sb = big.tile([128, kt, hc_out], F16)
            nc.sync.dma_start(wr_sb[:], wr[:].rearrange("(k p) n -> p k n", p=128))

            for t in range(NBLK):
                for w_sb, out_d in ((wl_sb, xl), (wr_sb, xr)):
                    acc = ps.tile([128, hc_out], F32, space="PSUM", tag="acc")
                    for kk in range(kt):
                        nc.tensor.matmul(
                            acc[:],
                            lhsT=hT_sb[:, kk, t * 128:(t + 1) * 128],
                            rhs=w_sb[:, kk, :],
                            start=(kk == 0), stop=(kk == kt - 1),
                        )
                    o = sb.tile([128, hc_out], odt, tag="o")
                    nc.scalar.copy(o[:], acc[:])
                    nc.sync.dma_start(out_d[t * 128:(t + 1) * 128, :], o[:])
    nc.compile()
    return nc


def build_edge1_hybrid(layer, s_blk, base, groups, act=None):
    """Layer-1 edge launch with the attention dot-product on the PE.

    Gathers xl/xr twice: edge-major (for aggregation) and transposed
    [c, e] (for scoring). logits = attT.T @ lrelu(xl_T + xr_T) accumulates
    over 4 c-tiles in PSUM; exp'd scores transpose back to edge-major via
    the PE for the scale + one-hot scatter-add matmuls.
    """
    f_in, H, C, hc, hcg, gdt = layer
    assert hc == 512 and H == 8 and gdt == F16
    nslot = NBLK * s_blk
    GMAX = 2 * (CH // 128)
    CT = hcg // 128  # 4 c-tiles

    nc = bacc.Bacc("TRN2", target_bir_lowering=False, debug=False, num_devices=NCORES)
    xl = nc.dram_tensor("xl", [N, hcg], gdt, kind="ExternalInput")
    xr = nc.dram_tensor("xr", [NPAD, hcg], gdt, kind="ExternalInput")
    src16 = nc.dram_tensor("src16", [128, nslot * (CH // 16)], I16, kind="ExternalInput")
    dst16 = nc.dram_tensor("dst16", [128, nslot * (CH // 16)], I16, kind="ExternalInput")
    dstloc = nc.dram_tensor("dstloc", [128, nslot * 4], F32, kind="ExternalInput")
    attT = nc.dram_tensor("attT", [128, CT, H], gdt, kind="ExternalInput")
    b_rep = nc.dram_tensor("b_rep", [128, hc], F32, kind="ExternalInput")
    iota_rep = nc.dram_tensor("iota_rep", [128, 128], F32, kind="ExternalInput")
    ident8 = nc.dram_tensor("ident8", [8, 8], F32, kind="ExternalInput")
    h_out = nc.dram_tensor("h_out", [NPAD, hc], F32, kind="ExternalOutput")

    with TileContext(nc) as tc:
        with (
            tc.tile_pool(name="cst", bufs=1) as cst,
            tc.tile_pool(name="g", bufs=2) as gp,
            tc.tile_pool(name="w", bufs=2) as wp,
            tc.tile_pool(name="o", bufs=2) as op_,
            tc.tile_pool(name="ps", bufs=2, space="PSUM") as ps,
            tc.tile_pool(name="ps2", bufs=2, space="PSUM") as ps2,
            tc.tile_pool(name="ps3", bufs=1, space="PSUM") as ps3,
        ):
            src16_sb = cst.tile([128, nslot * (CH // 16)], I16)
            nc.sync.dma_start(src16_sb[:], src16[:])
            dst16_sb = cst.tile([128, nslot * (CH // 16)], I16)
            nc.sync.dma_start(dst16_sb[:], dst16[:])
            dstloc_sb = cst.tile([128, nslot * 4], F32)
            nc.sync.dma_start(dstloc_sb[:], dstloc[:])
            attT_sb = cst.tile([128, CT, H], gdt)
            nc.sync.dma_start(attT_sb[:], attT[:])
            b_sb = cst.tile([128, hc], F32)
            nc.sync.dma_start(b_sb[:], b_rep[:])
            iota_sb = cst.tile([128, 128], F32)
            nc.sync.dma_start(iota_sb[:], iota_rep[:])
            id8_sb = cst.tile([8, 8], F32)
            nc.sync.dma_start(id8_sb[:], ident8[:])

            for b in range(NBLK):
                acc = ps.tile([128, hc], F32, space="PSUM", tag="acc")
                den = ps2.tile([128, H], F32, space="PSUM", tag="den")
                for gi, (s0, nsl) in enumerate(groups):
                    p = b * s_blk + s0
                    G = nsl * (CH // 128)
                    nidx = nsl * CH
                    nhalf = nidx // 512
                    xl_g = gp.tile([128, GMAX, hcg], gdt, tag="xl")
                    nc.gpsimd.dma_gather(
                        out_ap=xl_g[:, :G, :], in_ap=xl[int(base[p]):, :],
                        idxs_ap=src16_sb[:, p * 32:p * 32 + nsl * 32],
                        num_idxs=nidx, num_idxs_reg=nidx, elem_size=hcg,
                    )
                    xl_t = gp.tile([128, CT, nidx], gdt, tag="xlt")
                    nc.gpsimd.dma_gather(
                        out_ap=xl_t[:], in_ap=xl[int(base[p]):, :],
                        idxs_ap=src16_sb[:, p * 32:p * 32 + nsl * 32],
                        num_idxs=nidx, num_idxs_reg=nidx, elem_size=hcg,
                        transpose=True,
                    )
                    xr_t = gp.tile([128, CT, nidx], gdt, tag="xrt")
                    nc.gpsimd.dma_gather(
                        out_ap=xr_t[:], in_ap=xr[:],
                        idxs_ap=dst16_sb[:, p * 32:p * 32 + nsl * 32],
                        num_idxs=nidx, num_idxs_reg=nidx, elem_size=hcg,
                        transpose=True,
                    )
                    vt = wp.tile([128, CT, nidx], gdt, tag="vt")
                    nc.vector.tensor_add(vt[:], xl_t[:], xr_t[:])
                    lrt = wp.tile([128, CT, nidx], gdt, tag="lrt")
                    nc.scalar.activation(lrt[:], vt[:],
                                         act or mybir.ActivationFunctionType.Prelu,
                                         alpha=NEG_SLOPE)
                    lg = []
                    for hf in range(nhalf):
                        lgt = ps3.tile([8, 512], F32, space="PSUM", tag=f"lg{hf}",
                                       name=f"lg{hf}")
                        lg.append(lgt)
                    for ct in range(CT):
                        for hf in range(nhalf):
                            nc.tensor.matmul(
                                lg[hf][:], lhsT=attT_sb[:, ct, :],
                                rhs=lrt[:, ct, hf * 512:(hf + 1) * 512],
                                start=(ct == 0), stop=(ct == CT - 1))
                    lgs = wp.tile([8, 2 * CH], F32, tag="lgs")
                    for hf in range(nhalf):
                        nc.scalar.copy(lgs[:, hf * 512:(hf + 1) * 512], lg[hf][:])
                    un = wp.tile([128, GMAX, H], F32, tag="un")
                    for g in range(G):
                        tp = ps3.tile([128, H], F32, space="PSUM", tag="tp")
                        nc.tensor.transpose(tp[:], lgs[:, g * 128:(g + 1) * 128],
                                            id8_sb[:])
                        nc.scalar.activation(un[:, g, :], tp[:],
                                             mybir.ActivationFunctionType.Exp)
                    sc = wp.tile([128, GMAX, H, C], F32, tag="sc")
                    nc.vector.tensor_mul(
                        sc[:, :G, :, :],
                        xl_g[:, :G, :hc].rearrange("p g (h c) -> p g h c", h=H),
                        un[:, :G, :].unsqueeze(3).broadcast_to([128, G, H, C]))
                    for g in range(G):
                        oh = wp.tile([128, 128], F32, tag="oh")
                        nc.vector.tensor_scalar(
                            out=oh[:], in0=iota_sb[:],
                            scalar1=dstloc_sb[:, p * 4 + g:p * 4 + g + 1],
                            scalar2=None, op0=mybir.AluOpType.is_equal)
                        first = (gi == 0 and g == 0)
                        lastmm = (gi == len(groups) - 1 and g == G - 1)
                        nc.tensor.matmul(
                            acc[:], lhsT=oh[:],
                            rhs=sc[:, g, :, :].rearrange("p h c -> p (h c)"),
                            start=first, stop=lastmm)
                        nc.tensor.matmul(
                            den[:], lhsT=oh[:], rhs=un[:, g, :],
                            start=first, stop=lastmm)

                recip = op_.tile([128, H], F32, tag="recip")
                nc.vector.reciprocal(recip[:], den[:])
                o1 = op_.tile([128, H, C], F32, tag="o1")
                nc.vector.tensor_mul(
                    o1[:], acc[:].rearrange("p (h c) -> p h c", h=H),
                    recip[:].unsqueeze(2).broadcast_to([128, H, C]))
                o2 = op_.tile([128, hc], F32, tag="o2")
                nc.vector.tensor_add(o2[:], o1[:].rearrange("p h c -> p (h c)"), b_sb[:])
                h = op_.tile([128, hc], F32, tag="h")
                nc.scalar.activation(h[:], o2[:], mybir.ActivationFunctionType.Relu)
                nc.sync.dma_start(h_out[b * 128:(b + 1) * 128, :], h[:])
    nc.compile()
    return nc


def build_edge(layer, s_blk, base, groups, last=False):
    """Edge launch for one GATv2 layer (+ optional fc head for the last)."""
    f_in, H, C, hc, hcg, gdt = layer
    nslot = NBLK * s_blk
    GMAX = 2 * (CH // 128)
    # all-f16 aggregation path for layer 1 (exp biased by -2 so f16 can't
    # overflow; numerator and denominator share the scale so ratios are exact)
    f16agg = (gdt == F16 and hc == 512)
    adt = F16 if f16agg else F32
    ebias = -2.0 if f16agg else 0.0

    nc = bacc.Bacc("TRN2", target_bir_lowering=False, debug=False, num_devices=NCORES)
    xl = nc.dram_tensor("xl", [N, hcg], gdt, kind="ExternalInput")
    xr = nc.dram_tensor("xr", [NPAD, hcg], gdt, kind="ExternalInput")
    src16 = nc.dram_tensor("src16", [128, nslot * (CH // 16)], I16, kind="ExternalInput")
    dst16 = nc.dram_tensor("dst16", [128, nslot * (CH // 16)], I16, kind="ExternalInput")
    dstloc = nc.dram_tensor("dstloc", [128, nslot * 4], F32, kind="ExternalInput")
    att_rep = nc.dram_tensor("att_rep", [128, hc], gdt, kind="ExternalInput")
    b_rep = nc.dram_tensor("b_rep", [128, hc], F32, kind="ExternalInput")
    iota_rep = nc.dram_tensor("iota_rep", [128, 128], adt, kind="ExternalInput")
    h_out = nc.dram_tensor("h_out", [NPAD, hc], F32, kind="ExternalOutput")
    if last:
        fc1_w = nc.dram_tensor("fc1_w", [16, 8], F32, kind="ExternalInput")
        fc1_b = nc.dram_tensor("fc1_b", [8, 1], F32, kind="ExternalInput")
        fc2_w = nc.dram_tensor("fc2_w", [8, 2], F32, kind="ExternalInput")
        fc2_b = nc.dram_tensor("fc2_b", [2, 1], F32, kind="ExternalInput")
        outT = nc.dram_tensor("outT", [2, NPAD], F32, kind="ExternalOutput")

    with TileContext(nc) as tc:
        nb = 3 if hc >= 512 else 4
        with (
            tc.tile_pool(name="cst", bufs=1) as cst,
            tc.tile_pool(name="g", bufs=nb) as gp,
            tc.tile_pool(name="w", bufs=nb) as wp,
            tc.tile_pool(name="o", bufs=2) as op_,
            tc.tile_pool(name="ps", bufs=2, space="PSUM") as ps,
            tc.tile_pool(name="ps2", bufs=2, space="PSUM") as ps2,
            tc.tile_pool(name="ps3", bufs=1, space="PSUM") as ps3,
        ):
            src16_sb = cst.tile([128, nslot * (CH // 16)], I16)
            nc.sync.dma_start(src16_sb[:], src16[:])
            dst16_sb = cst.tile([128, nslot * (CH // 16)], I16)
            nc.sync.dma_start(dst16_sb[:], dst16[:])
            dstloc_sb = cst.tile([128, nslot * 4], F32)
            nc.sync.dma_start(dstloc_sb[:], dstloc[:])
            att_sb = cst.tile([128, hc], gdt)
            nc.sync.dma_start(att_sb[:], att_rep[:])
            b_sb = cst.tile([128, hc], F32)
            nc.sync.dma_start(b_sb[:], b_rep[:])
            iota_sb = cst.tile([128, 128], adt)
            nc.sync.dma_start(iota_sb[:], iota_rep[:])
            ebias_sb = None
            if f16agg:
                ebias_sb = cst.tile([128, 1], F32)
                nc.vector.memset(ebias_sb[:], ebias)
            if last:
                fc1w_sb = cst.tile([16, 8], F32)
                nc.sync.dma_start(fc1w_sb[:], fc1_w[:])
                fc1b_sb = cst.tile([8, 1], F32)
                nc.sync.dma_start(fc1b_sb[:], fc1_b[:])
                fc2w_sb = cst.tile([8, 2], F32)
                nc.sync.dma_start(fc2w_sb[:], fc2_w[:])
                fc2b_sb = cst.tile([2, 1], F32)
                nc.sync.dma_start(fc2b_sb[:], fc2_b[:])
                ident = cst.tile([128, 128], F32)
                make_identity(nc, ident[:])

            for b in range(NBLK):
                acc = ps.tile([128, hc], F32, space="PSUM", tag="acc")
                den = ps2.tile([128, H], F32, space="PSUM", tag="den")
                for gi, (s0, nsl) in enumerate(groups):
                    p = b * s_blk + s0           # first slot of the gather unit
                    G = nsl * (CH // 128)        # 128-edge subchunks in this unit
                    nidx = nsl * CH
                    xl_g = gp.tile([128, GMAX, hcg], gdt, tag="xl")
                    nc.gpsimd.dma_gather(
                        out_ap=xl_g[:, :G, :], in_ap=xl[int(base[p]):, :],
                        idxs_ap=src16_sb[:, p * 32:p * 32 + nsl * 32],
                        num_idxs=nidx, num_idxs_reg=nidx, elem_size=hcg,
                    )
                    xr_g = gp.tile([128, GMAX, hcg], gdt, tag="xr")
                    nc.gpsimd.dma_gather(
                        out_ap=xr_g[:, :G, :], in_ap=xr[:],
                        idxs_ap=dst16_sb[:, p * 32:p * 32 + nsl * 32],
                        num_idxs=nidx, num_idxs_reg=nidx, elem_size=hcg,
                    )
                    v = wp.tile([128, GMAX, hc], gdt, tag="v")
                    nc.vector.tensor_add(v[:, :G, :], xl_g[:, :G, :hc], xr_g[:, :G, :hc])
                    lr = wp.tile([128, GMAX, hc], gdt, tag="lr")
                    nc.scalar.activation(lr[:, :G, :], v[:, :G, :],
                                         mybir.ActivationFunctionType.Prelu,
                                         alpha=NEG_SLOPE)
                    t4 = wp.tile([128, GMAX, H, C], gdt, tag="t4")
                    nc.vector.tensor_mul(
                        t4[:, :G, :, :],
                        lr[:, :G, :].rearrange("p g (h c) -> p g h c", h=H),
                        att_sb[:].rearrange("p (h c) -> p h c", h=H)
                              .unsqueeze(1).broadcast_to([128, G, H, C]))
                    logit = wp.tile([128, GMAX, H], F32, tag="logit")
                    if f16agg:
                        # fold C in half with a 2x-packed f16 add, then reduce
                        # half the elements at the DVE's 1x reduce rate
                        th = wp.tile([128, GMAX, H, C // 2], gdt, tag="th")
                        nc.vector.tensor_add(th[:, :G, :, :],
                                             t4[:, :G, :, :C // 2],
                                             t4[:, :G, :, C // 2:])
                        nc.vector.tensor_reduce(logit[:, :G, :], th[:, :G, :, :],
                                                axis=mybir.AxisListType.X,
                                                op=mybir.AluOpType.add)
                    else:
                        nc.vector.tensor_reduce(logit[:, :G, :], t4[:, :G, :, :],
                                                axis=mybir.AxisListType.X,
                                                op=mybir.AluOpType.add)
                    un = wp.tile([128, GMAX, H], adt, tag="un")
                    nc.scalar.activation(un[:, :G, :], logit[:, :G, :],
                                         mybir.ActivationFunctionType.Exp,
                                         bias=(ebias_sb[:] if f16agg else 0.0))
                    sc = wp.tile([128, GMAX, H, C], adt, tag="sc")
                    xl4 = xl_g[:, :G, :hc].rearrange("p g (h c) -> p g h c", h=H)
                    un4 = un[:, :G, :].unsqueeze(3).broadcast_to([128, G, H, C])
                    if f16agg:
                        # split the 1x broadcast multiply between the DVE and
                        # the otherwise-idle GPSIMD engine (by head ranges)
                        HS = 4
                        nc.vector.tensor_mul(
                            sc[:, :G, :HS, :], xl4[:, :, :HS, :], un4[:, :, :HS, :])
                        nc.gpsimd.tensor_mul(
                            sc[:, :G, HS:, :], xl4[:, :, HS:, :], un4[:, :, HS:, :])
                    else:
                        nc.vector.tensor_mul(sc[:, :G, :, :], xl4, un4)
                    for g in range(G):
                        oh = wp.tile([128, 128], adt, tag="oh")
                        nc.vector.tensor_scalar(
                            out=oh[:], in0=iota_sb[:],
                            scalar1=dstloc_sb[:, p * 4 + g:p * 4 + g + 1],
                            scalar2=None, op0=mybir.AluOpType.is_equal)
                        first = (gi == 0 and g == 0)
                        lastmm = (gi == len(groups) - 1 and g == G - 1)
                        nc.tensor.matmul(
                            acc[:], lhsT=oh[:],
                            rhs=sc[:, g, :, :].rearrange("p h c -> p (h c)"),
                            start=first, stop=lastmm)
                        nc.tensor.matmul(
                            den[:], lhsT=oh[:], rhs=un[:, g, :],
                            start=first, stop=lastmm)

                # normalize + bias + relu
                recip = op_.tile([128, H], F32, tag="recip")
                nc.vector.reciprocal(recip[:], den[:])
                o1 = op_.tile([128, H, C], F32, tag="o1")
                nc.vector.tensor_mul(
                    o1[:], acc[:].rearrange("p (h c) -> p h c", h=H),
                    recip[:].unsqueeze(2).broadcast_to([128, H, C]))
                o2 = op_.tile([128, hc], F32, tag="o2")
                nc.vector.tensor_add(o2[:], o1[:].rearrange("p h c -> p (h c)"), b_sb[:])
                h = op_.tile([128, hc], F32, tag="h")
                nc.scalar.activation(h[:], o2[:], mybir.ActivationFunctionType.Relu)
                nc.sync.dma_start(h_out[b * 128:(b + 1) * 128, :], h[:])

                if last:
                    # emb^T via PE transpose, then fc head
                    tp = ps3.tile([16, 128], F32, space="PSUM", tag="tp")
                    nc.tensor.transpose(tp[:], h[:, :16], ident[:])
                    embT = op_.tile([16, 128], F32, tag="embT")
                    nc.scalar.copy(embT[:], tp[:])
                    p1 = ps3.tile([8, 128], F32, space="PSUM", tag="p1")
                    nc.tensor.matmul(p1[:], lhsT=fc1w_sb[:], rhs=embT[:],
                                     start=True, stop=True)
                    a1 = op_.tile([8, 128], F32, tag="a1")
                    nc.scalar.activation(a1[:], p1[:], mybir.ActivationFunctionType.Relu,
                                         bias=fc1b_sb[:])
                    p2 = ps3.tile([2, 128], F32, space="PSUM", tag="p2")
                    nc.tensor.matmul(p2[:], lhsT=fc2w_sb[:], rhs=a1[:],
                                     start=True, stop=True)
                    a2 = op_.tile([2, 128], F32, tag="a2")
                    nc.scalar.activation(a2[:], p2[:], mybir.ActivationFunctionType.Identity,
                                         bias=fc2b_sb[:])
                    nc.sync.dma_start(outT[:, b * 128:(b + 1) * 128], a2[:])
    nc.compile()
    return nc


# ----------------------------------------------------------------------------
# Orchestration
# ----------------------------------------------------------------------------

def _rep(v, hc):
    return np.ascontiguousarray(np.tile(np.asarray(v, np.float32).reshape(1, hc), (128, 1)))


def kernel(x, edge_index, Wl1, Wr1, att1, b1, Wl2, Wr2, att2, b2,
           Wl3, Wr3, att3, b3, fc1_w, fc1_b, fc2_w, fc2_b):
    x = np.asarray(x, np.float32)
    ep = prep_edges(np.asarray(edge_index))
    s_blk, base = ep["s_blk"], ep["base"]
    iota = np.ascontiguousarray(np.tile(np.arange(128, dtype=np.float32)[None, :], (128, 1)))

    cores = list(range(NCORES))
    Ws = [(np.asarray(Wl1, np.float32), np.asarray(Wr1, np.float32)),
          (np.asarray(Wl2, np.float32), np.asarray(Wr2, np.float32)),
          (np.asarray(Wl3, np.float32), np.asarray(Wr3, np.float32))]
    atts = [np.asarray(att1, np.float32), np.asarray(att2, np.float32),
            np.asarray(att3, np.float32)]
    bs = [np.asarray(b1, np.float32), np.asarray(b2, np.float32),
          np.asarray(b3, np.float32)]

    h = x  # full node features [N, f_in]
    emb = None
    outT_sh = None
    for li, layer in enumerate(LAYERS):
        f_in, H, C, hc, hcg, gdt = layer
        npdt = np.float16 if gdt == F16 else np.float32
        wl, wr = Ws[li]
        if hcg != hc:  # pad weight cols for gather alignment (layer 3)
            wl = np.concatenate([wl, np.zeros((f_in, hcg - hc), np.float32)], 1)
            wr = np.concatenate([wr, np.zeros((f_in, hcg - hc), np.float32)], 1)

        # --- mm launch: per-core shard transform ---
        import sys
        print(f"[kernel] mm{li+1}", file=sys.stderr)
        nc_mm = build_mm(f_in, hcg, odt=gdt)
        in_maps = []
        wl16, wr16 = wl.astype(np.float16), wr.astype(np.float16)
        for k in cores:
            hsh = np.zeros((NPAD, f_in), np.float16)
            hsh[:NSH] = h[k * NSH:(k + 1) * NSH]
            in_maps.append({"hT": np.ascontiguousarray(hsh.T), "wl": wl16, "wr": wr16})
        res = run_bass_kernel_spmd(nc_mm, in_maps, core_ids=cores).results
        xl_full = np.concatenate([r["xl"][:NSH] for r in res], 0)  # [N, hcg]
        xr_shs = [r["xr"] for r in res]

        # --- edge launch ---
        print(f"[kernel] edge{li+1}", file=sys.stderr)
        # transposed dma_gather (needed by the hybrid scorer) faults on this
        # axon terminal's ucode -- keep the edge-major path everywhere.
        hybrid = False
        if hybrid:
            nc_e = build_edge1_hybrid(layer, s_blk, base, ep["groups"])
        else:
            nc_e = build_edge(layer, s_blk, base, ep["groups"], last=(li == 2))
        adt = np.float16 if (gdt == F16 and hc == 512) else np.float32
        in_maps = []
        for k in cores:
            m = {
                "xl": xl_full, "xr": xr_shs[k],
                "src16": ep["src16"][k], "dst16": ep["dst16"][k],
                "dstloc": ep["dstloc"][k],
                "b_rep": _rep(bs[li], hc), "iota_rep": iota.astype(adt),
            }
            if hybrid:
                A = np.zeros((hc, H), np.float32)
                af = atts[li].reshape(-1)
                for i in range(hc):
                    A[i, i // C] = af[i]
                m["attT"] = np.ascontiguousarray(
                    A.reshape(hc // 128, 128, H).transpose(1, 0, 2)).astype(npdt)
                m["ident8"] = np.eye(8, dtype=np.float32)
            else:
                m["att_rep"] = _rep(atts[li].reshape(-1), hc).astype(npdt)
            if li == 2:
                m.update({
                    "fc1_w": np.ascontiguousarray(np.asarray(fc1_w, np.float32)),
                    "fc1_b": np.asarray(fc1_b, np.float32).reshape(8, 1).copy(),
                    "fc2_w": np.ascontiguousarray(np.asarray(fc2_w, np.float32)),
                    "fc2_b": np.asarray(fc2_b, np.float32).reshape(2, 1).copy(),
                })
            in_maps.append(m)
        res = run_bass_kernel_spmd(nc_e, in_maps, core_ids=cores).results
        h = np.concatenate([r["h_out"][:NSH] for r in res], 0)  # [N, hc]
        if li == 2:
            emb = h
            outT_sh = [r["outT"] for r in res]

    out = np.concatenate([t.T[:NSH] for t in outT_sh], 0)  # [N, 2]
    return out, emb
